# revision 1
# baseline (speedup 1.0000x reference)
"""Trainium2 Bass kernel for nn_AggAtt (DCNv2-style deformable conv block).

Math (simplified from the reference):
  - om = conv3x3(inp, w_om) + b_om  -> off = om[:2], masks m0..m4 = sigmoid(om[2:7])
  - Only 5 of 9 DCN taps have nonzero mask. Base offset cancels with the conv
    grid, so tap sample positions are simply:
      tap0: (y - h/2, x - w/2) m0 | tap2: (y - h/2, x + w/2) m1
      tap4: (y + off0, x + off1) m2 | tap6: (y + h/2, x - w/2) m3
      tap8: (y,       x + w/2) m4
  - feat[o] = sum_{c,tap} W[o,c,tap] * bilinear(inp[c], pos_tap) * m_tap
  - out = conv1x1(relu(feat + bias)) + b_1x1

Strategy: 8 cores, each handles (batch b, 32 output rows) with an 8-row halo.
Host pre-builds the channels-last bf16 gather source (clast) and the bf16
channel-major slice (xinb), so the device only: computes om (PE), per-pixel
coords/weights (DVE, pixel-major, per half-core group), one merged dma_gather
per 512-px chunk, then combines corners via diag-scaled PE matmuls
(out = G^T @ diag(alpha), accumulated in PSUM per tap), drains to SBUF
(ACT), and runs the 5-tap einsum + ReLU + 1x1 conv.
"""

import os
from contextlib import ExitStack

import numpy as np
import ml_dtypes

import concourse.bass as bass
import concourse.mybir as mybir
import concourse.tile as tile
from concourse import bacc
from concourse.bass_utils import run_bass_kernel_spmd

BF16 = mybir.dt.bfloat16
F32 = mybir.dt.float32
FP8 = mybir.dt.float8e4
I16 = mybir.dt.int16
ALU = mybir.AluOpType
ACTF = mybir.ActivationFunctionType

QFP8 = bool(int(os.environ.get("QFP8", "0")))  # fp8 gather source + om input
GDT = FP8 if QFP8 else BF16
OM_WSCALE = 64.0  # keep fp8 om weights out of e4m3 subnormals

B, C, H, W = 2, 256, 128, 128
O, F = 256, 2
NCORES = 8
RPC = H * B // NCORES          # 32 output rows per core
HALO = 8
RTOT = RPC + 2 * HALO          # 48 input rows per core slice
NPX = RPC * W                  # 4096 output pixels per core
NSRC = RTOT * W                # 6144 source pixels per core
CHUNK = 512                    # pixels per pipeline chunk (4 rows)
NCHUNK = NPX // CHUNK          # 8
G = RPC                        # free dim of pixel-major field tiles (32)
GH = G // 2                    # rows per coord group (half-core)
# round-to-int magic: 1.5*2^23 keeps x+MAGIC in the 1.0-ulp range of fp32
# for |x| < 2^22, so (x + MAGIC) - MAGIC == round-to-nearest-integer(x)
MAGIC = float(3 * 2 ** 22)

# merged-gather element list: (tap-slot κi 0..4, row a/b). tap indices into
# masks m0..m4 and W taps [0,2,4,6,8]; pair id: A=x-w/2, B=x+w/2, C=x+off1
KR = [(0, 'a'), (0, 'b'), (1, 'a'), (1, 'b'), (2, 'a'), (2, 'b'),
      (3, 'a'), (3, 'b'), (4, 'a')]
TAP_PAIR = ['A', 'B', 'C', 'A', 'B']   # colpair per tap-slot
NKR = len(KR)                           # 9
# corners per tap-slot: list of (element index i, column s)
TAP_CORNERS = [[] for _ in range(5)]
for _i, (_ki, _) in enumerate(KR):
    TAP_CORNERS[_ki] += [(_i, 0), (_i, 1)]


def build(nc: bass.Bass):
    # ---- I/O ----
    xinb = nc.dram_tensor("xinb", [2, 128, RTOT, 128], GDT, kind="ExternalInput")
    clast_h = nc.dram_tensor("clast", [NSRC, 256], GDT, kind="ExternalInput")
    whin = nc.dram_tensor("whin", [2, RPC, 128], F32, kind="ExternalInput")
    ygl_in = nc.dram_tensor("ygl", [1, G], F32, kind="ExternalInput")
    rb_in = nc.dram_tensor("rbase", [1, 1], F32, kind="ExternalInput")
    womT = nc.dram_tensor("womT", [128, 9, 2, 7] if QFP8 else [128, 18, 7],
                          GDT, kind="ExternalInput")
    wmainT = nc.dram_tensor("wmainT", [128, 5, 2, 2, 128], BF16, kind="ExternalInput")
    w1x1T = nc.dram_tensor("w1x1T", [128, 2, 2], BF16, kind="ExternalInput")
    bom_in = nc.dram_tensor("bom", [7, 1], F32, kind="ExternalInput")
    bmain_in = nc.dram_tensor("bmain", [128, 2], F32, kind="ExternalInput")
    b1_in = nc.dram_tensor("b1", [2, 1], F32, kind="ExternalInput")
    out = nc.dram_tensor("out", [2, RPC, 128], F32, kind="ExternalOutput")

    ident_h = nc.inline_tensor(np.eye(128, dtype=ml_dtypes.bfloat16), "ident")
    identf_h = nc.inline_tensor(np.eye(128, dtype=np.float32), "identf")
    xcol_h = nc.inline_tensor(
        np.arange(128, dtype=np.float32).reshape(128, 1), "xcol")
    # permutation matrix: perm[p, j] = 1 iff p == 16*(j%8) + j//8, so a
    # matmul in.T @ perm reorders pixel columns as j = q*8 + r (p = 16r+q)
    _pm = np.zeros((128, 128), np.float32)
    _jj = np.arange(128)
    _pm[16 * (_jj % 8) + _jj // 8, _jj] = 1.0
    perm_h = nc.inline_tensor(_pm, "perm")

    # ---- internal DRAM scratch for gather idx layout ----
    d4 = nc.dram_tensor("d4", [NCHUNK, NKR * 32 * 16], I16)

    with tile.TileContext(nc) as tc, ExitStack() as ctx:
        P = ctx.enter_context
        singles = P(tc.tile_pool(name="singles", bufs=1))
        ppad = P(tc.tile_pool(name="ppad", bufs=1))
        pom = P(tc.tile_pool(name="pom", bufs=2))
        pfields = P(tc.tile_pool(name="pfields", bufs=1))
        pcoord = P(tc.tile_pool(name="pcoord", bufs=1))
        pg = P(tc.tile_pool(name="pg", bufs=2))
        pdiag = P(tc.tile_pool(name="pdiag", bufs=20))
        psc = P(tc.tile_pool(name="psc", bufs=2))
        pfeat = P(tc.tile_pool(name="pfeat", bufs=2))
        pout = P(tc.tile_pool(name="pout", bufs=2))
        # PSUM pools
        ps_small = P(tc.tile_pool(name="ps_small", bufs=2, space="PSUM"))
        ps_S = P(tc.tile_pool(name="ps_S", bufs=2, space="PSUM"))
        ps_f = P(tc.tile_pool(name="ps_f", bufs=2, space="PSUM"))
        ps_idx = P(tc.tile_pool(name="ps_idx", bufs=2, space="PSUM"))

        # ---- load constants / weights ----
        ident = singles.tile([128, 128], BF16, name="ident_sb")
        nc.sync.dma_start(ident[:], ident_h[:])
        identf = singles.tile([128, 128], F32, name="identf_sb")
        nc.sync.dma_start(identf[:], identf_h[:])
        xcol = singles.tile([128, 1], F32, name="xcol_sb")
        nc.sync.dma_start(xcol[:], xcol_h[:])
        perm = singles.tile([128, 128], F32, name="perm_sb")
        nc.sync.dma_start(perm[:], perm_h[:])
        wom_sb = singles.tile([128, 9, 2, 7] if QFP8 else [128, 18, 7], GDT,
                              name="wom_sb")
        nc.sync.dma_start(wom_sb[:], womT[:])
        wmain_sb = singles.tile([128, 5, 2, 2, 128], BF16, name="wmain_sb")
        nc.sync.dma_start(wmain_sb[:], wmainT[:])
        w1_sb = singles.tile([128, 2, 2], BF16, name="w1_sb")
        nc.sync.dma_start(w1_sb[:], w1x1T[:])
        bom_sb = singles.tile([7, 1], F32, name="bom_sb")
        nc.sync.dma_start(bom_sb[:], bom_in[:])
        bmain_sb = singles.tile([128, 2], F32, name="bmain_sb")
        nc.sync.dma_start(bmain_sb[:], bmain_in[:])
        b1_sb = singles.tile([2, 1], F32, name="b1_sb")
        nc.sync.dma_start(b1_sb[:], b1_in[:])
        rbase = singles.tile([128, 1], F32, name="rbase_sb")
        nc.sync.dma_start(
            rbase[:],
            bass.AP(tensor=rb_in, offset=0, ap=[[0, 128], [1, 1]]))
        ygl = singles.tile([128, G], F32, name="ygl_sb")
        nc.sync.dma_start(
            ygl[:],
            bass.AP(tensor=ygl_in, offset=0, ap=[[0, 128], [1, G]]))
        # [row%16, half, c2, x] so half-core slices keep base partition 0
        wh_sb = singles.tile([16, 2, 2, 128], F32, name="wh_sb")
        for h in range(2):
            nc.sync.dma_start(
                wh_sb[:, h, :, :],
                bass.AP(tensor=whin, offset=h * 16 * 128,
                        ap=[[128, 16], [32 * 128, 2], [1, 128]]))

        # ---- input slice (host-prepped), padded cols (130 wide); load in
        # row-halves so om chunk 0 can start before the whole slice lands ----
        inp_pad = ppad.tile([128, 2, RTOT, 130], GDT, name="inp_pad")
        for ct in range(2):
            nc.vector.memset(inp_pad[:, ct, :, 0:1], 0.0)
            nc.vector.memset(inp_pad[:, ct, :, 129:130], 0.0)
        for rh in range(2):
            rs = slice(rh * (RTOT // 2), (rh + 1) * (RTOT // 2))
            for ct in range(2):
                nc.sync.dma_start(inp_pad[:, ct, rs, 1:129],
                                  xinb[ct, :, rs, :])

        # ---- whole-core field tiles ----
        F_om = pfields.tile([128, G, 7], F32, name="F_om")
        F_wh = pfields.tile([128, G, 2], F32, name="F_wh")
        Fidx2 = pfields.tile([128, NCHUNK, 64], F32, name="Fidx2")
        nc.vector.memset(Fidx2[:], 0.0)
        idx_sb = pfields.tile([128, NCHUNK, NKR * 32], I16, name="idx_sb")
        Fal = pfields.tile([128, G, 2 * NKR], F32, name="Fal")
        TpI = pfields.tile([128, 4, 128], I16, name="TpI")

        # funnel rbase through a DVE copy so ops reading it alongside another
        # DMA-loaded tile don't exceed the per-instruction sync-wait limit
        rbase2 = pcoord.tile([128, 1], F32, name="rbase2", tag="rbase2")
        nc.vector.tensor_copy(rbase2[:], rbase[:])
        rbase = rbase2

        def om_chunk(ch):
            pso = ps_small.tile([7, 512], F32, name="pso", tag="pso")
            if QFP8:
                t = 0
                for dy in range(3):
                    for dx in range(3):
                        rhs = inp_pad[:, :, ch * 4 + dy + 7:ch * 4 + dy + 11,
                                      dx:dx + 128]
                        nc.tensor.matmul(
                            pso[:], wom_sb[:, t, :, :], rhs,
                            start=(t == 0), stop=(t == 8),
                            perf_mode=mybir.MatmulPerfMode.DoubleRow)
                        t += 1
            else:
                t = 0
                for dy in range(3):
                    for dx in range(3):
                        for ct in range(2):
                            rhs = inp_pad[:, ct,
                                          ch * 4 + dy + 7:ch * 4 + dy + 11,
                                          dx:dx + 128]
                            nc.tensor.matmul(
                                pso[:], wom_sb[:, t, :], rhs,
                                start=(t == 0), stop=(t == 17))
                            t += 1
            om_sb = pom.tile([7, 512], F32, name="om_sb", tag="om_sb")
            if QFP8:
                nc.scalar.activation(om_sb[:], pso[:], ACTF.Identity,
                                     bias=bom_sb[:], scale=1.0 / OM_WSCALE)
            else:
                nc.scalar.activation(om_sb[:], pso[:], ACTF.Identity,
                                     bias=bom_sb[:])
            for gg in range(4):
                pfo = ps_idx.tile([128, 128], F32, name="pfo", tag="pidx")
                nc.tensor.matmul(
                    pfo[:, 0:7], om_sb[:, gg * 128:(gg + 1) * 128],
                    identf[0:7, 0:7], is_transpose=True,
                    start=True, stop=True)
                nc.vector.tensor_copy(F_om[:, ch * 4 + gg, :], pfo[:, 0:7])

        def T(name, h, sz=GH):
            return pcoord.tile([128, sz], F32, name=name, tag=f"{name}{h}")

        def floor_of(src, pfx, h, sz=GH):
            r = T(pfx + "_r", h, sz)
            nc.vector.tensor_scalar(r[:], src[:], MAGIC, MAGIC, ALU.add,
                                    ALU.subtract)
            g = T(pfx + "_g", h, sz)
            nc.vector.tensor_tensor(g[:], r[:], src[:], ALU.is_gt)
            f = T(pfx + "_f", h, sz)
            nc.vector.tensor_tensor(f[:], r[:], g[:], ALU.subtract)
            return f

        def yrows_idx(dy_ap, pfx, h, sl=None, sz=GH):
            """index-path rows (a,b): returns (ys, y0, locs, rcs, ybs)"""
            if sl is None:
                sl = slice(GH * h, GH * (h + 1))
            ys = T(pfx + "_ys", h, sz)
            nc.vector.tensor_tensor(ys[:], ygl[:, sl], dy_ap, ALU.add)
            y0 = floor_of(ys, pfx + "_y0", h, sz)
            locs, rcs, ybs = [], [], []
            for nm, base in (("a", y0), ("b", None)):
                yb = base
                if yb is None:
                    yb = T(pfx + "_y1", h, sz)
                    nc.vector.tensor_scalar(yb[:], y0[:], 1.0, None, ALU.add)
                rc = T(pfx + "_rc" + nm, h, sz)
                nc.vector.tensor_scalar(rc[:], yb[:], 0.0, 127.0, ALU.max,
                                        ALU.min)
                loc = T(pfx + "_loc" + nm, h, sz)
                nc.vector.tensor_scalar(loc[:], rc[:], rbase[:], float(HALO),
                                        ALU.subtract, ALU.add)
                locs.append(loc); rcs.append(rc); ybs.append(yb)
            return ys, y0, locs, rcs, ybs

        def yrows_w(state, pfx, h, sz=GH):
            """alpha-path rows: returns (vA, vB)"""
            ys, y0, locs, rcs, ybs = state
            fy = T(pfx + "_fy", h, sz)
            nc.vector.tensor_tensor(fy[:], ys[:], y0[:], ALU.subtract)
            w0 = T(pfx + "_w0", h, sz)
            nc.vector.tensor_scalar(w0[:], fy[:], -1.0, 1.0, ALU.mult, ALU.add)
            vs = []
            for k, (nm, wgt) in enumerate((("a", w0), ("b", fy))):
                eq = T(pfx + "_eq" + nm, h, sz)
                nc.vector.tensor_tensor(eq[:], rcs[k][:], ybs[k][:],
                                        ALU.is_equal)
                v = T(pfx + "_v" + nm, h, sz)
                nc.vector.tensor_tensor(v[:], wgt[:], eq[:], ALU.mult)
                vs.append(v)
            return vs

        def xpair_idx(dx_ap, pfx, h, sz=GH):
            """index-path colpair: returns (xs, x0, xi)"""
            xs = T(pfx + "_xs", h, sz)
            nc.vector.tensor_scalar(xs[:], dx_ap, xcol[:], None, ALU.add)
            x0 = floor_of(xs, pfx + "_x0", h, sz)
            xi = T(pfx + "_xi", h, sz)
            nc.vector.tensor_scalar(xi[:], x0[:], 0.0, 126.0, ALU.max, ALU.min)
            return xs, x0, xi

        def xpair_w(state, pfx, h, sz=GH):
            """alpha-path colpair: returns (u0, u1)"""
            xs, x0, xi = state
            fx = T(pfx + "_fx", h, sz)
            nc.vector.tensor_tensor(fx[:], xs[:], x0[:], ALU.subtract)
            w0 = T(pfx + "_w0", h, sz)
            nc.vector.tensor_scalar(w0[:], fx[:], -1.0, 1.0, ALU.mult, ALU.add)
            mid = T(pfx + "_mid", h, sz)
            nc.vector.tensor_tensor(mid[:], xi[:], x0[:], ALU.is_equal)
            em1 = T(pfx + "_em1", h, sz)
            nc.vector.tensor_scalar(em1[:], x0[:], -1.0, None, ALU.is_equal)
            e127 = T(pfx + "_e127", h, sz)
            nc.vector.tensor_scalar(e127[:], x0[:], 127.0, None, ALU.is_equal)
            t1 = T(pfx + "_t1", h, sz)
            nc.vector.tensor_tensor(t1[:], w0[:], mid[:], ALU.mult)
            t2 = T(pfx + "_t2", h, sz)
            nc.vector.tensor_tensor(t2[:], fx[:], em1[:], ALU.mult)
            u0 = T(pfx + "_u0", h, sz)
            nc.vector.tensor_tensor(u0[:], t1[:], t2[:], ALU.add)
            t3 = T(pfx + "_t3", h, sz)
            nc.vector.tensor_tensor(t3[:], fx[:], mid[:], ALU.mult)
            t4 = T(pfx + "_t4", h, sz)
            nc.vector.tensor_tensor(t4[:], w0[:], e127[:], ALU.mult)
            u1 = T(pfx + "_u1", h, sz)
            nc.vector.tensor_tensor(u1[:], t3[:], t4[:], ALU.add)
            return u0, u1

        def coords_idx(h):
            """index-path only: everything the gather idxs depend on."""
            sl = slice(GH * h, GH * (h + 1))
            wF = F_wh[:, sl, 0]
            hF = F_wh[:, sl, 1]
            # F_wh for this group via PE transpose of wh rows
            for c2 in range(2):
                pfw = ps_idx.tile([128, 128], F32, name="pfw", tag="pidx")
                nc.tensor.matmul(pfw[:, 0:GH], wh_sb[0:GH, h, c2, :],
                                 identf[0:GH, 0:GH], is_transpose=True,
                                 start=True, stop=True)
                nc.vector.tensor_copy(F_wh[:, sl, c2], pfw[:, 0:GH])

            nh2 = T("nh2", h)
            nc.vector.tensor_scalar(nh2[:], hF, -0.5, None, ALU.mult)
            ph2 = T("ph2", h)
            nc.vector.tensor_scalar(ph2[:], hF, 0.5, None, ALU.mult)
            nw2 = T("nw2", h)
            nc.vector.tensor_scalar(nw2[:], wF, -0.5, None, ALU.mult)
            pw2 = T("pw2", h)
            nc.vector.tensor_scalar(pw2[:], wF, 0.5, None, ALU.mult)

            yT = yrows_idx(nh2[:], "yT", h)          # taps 0,1 (top)
            yB = yrows_idx(ph2[:], "yB", h)          # tap 3 (bottom)
            loc8 = T("loc8", h)
            nc.vector.tensor_scalar(loc8[:], ygl[:, sl], rbase[:], float(HALO),
                                    ALU.subtract, ALU.add)
            xA = xpair_idx(nw2[:], "xA", h)
            xB = xpair_idx(pw2[:], "xB", h)
            st = {'yT': yT, 'yB': yB, 'loc8': loc8, 'A': xA, 'B': xB}

            def emit_stt(i, loc, xi):
                nc.vector.scalar_tensor_tensor(
                    Fidx2[:, 4 * h:4 * h + 4, 4 * i:4 * i + 4],
                    loc[:].rearrange("p (c g) -> p c g", g=4), 128.0,
                    xi[:].rearrange("p (c g) -> p c g", g=4),
                    ALU.mult, ALU.add)

            taplocs = {0: yT[2], 1: yT[2], 3: yB[2], 4: [loc8, loc8]}
            for i, (ki, rab) in enumerate(KR):
                if ki == 2:
                    continue  # tap4 waits on om
                emit_stt(i, taplocs[ki][0 if rab == 'a' else 1],
                         st[TAP_PAIR[ki]][2])
            st['emit_stt'] = emit_stt
            return st

        def coords_idx_om(h, st, q):
            """tap4's index path for quarter q (depends on om chunks only
            of that quarter, so gathers can start after 2 om chunks)."""
            slq = slice(GH * h + 8 * q, GH * h + 8 * (q + 1))
            hq = f"{h}_{q}"
            y4 = yrows_idx(F_om[:, slq, 0], "y4", hq, sl=slq, sz=8)
            xC = xpair_idx(F_om[:, slq, 1], "xC", hq, sz=8)
            st['y4', q] = y4
            st['C', q] = xC
            for i, (ki, rab) in enumerate(KR):
                if ki != 2:
                    continue
                loc = y4[2][0 if rab == 'a' else 1]
                xi = xC[2]
                nc.vector.scalar_tensor_tensor(
                    Fidx2[:, 4 * h + 2 * q:4 * h + 2 * q + 2,
                          4 * i:4 * i + 4],
                    loc[:].rearrange("p (c g) -> p c g", g=4), 128.0,
                    xi[:].rearrange("p (c g) -> p c g", g=4),
                    ALU.mult, ALU.add)

        def coords_alpha(h, st):
            """alpha-path: bilinear weights * masks -> Fal (post-idx)."""
            sl = slice(GH * h, GH * (h + 1))
            vT = yrows_w(st['yT'], "yT", h)
            vB = yrows_w(st['yB'], "yB", h)
            uA = xpair_w(st['A'], "xA", h)
            uB = xpair_w(st['B'], "xB", h)
            tapv = [vT, vT, None, vB, [None, None]]
            tapu = {'A': uA, 'B': uB}
            for i, (ki, rab) in enumerate(KR):
                if ki == 2:  # tap4: per-quarter state
                    for q in range(2):
                        hq = f"{h}_{q}"
                        slq = slice(GH * h + 8 * q, GH * h + 8 * (q + 1))
                        v4 = yrows_w(st['y4', q], "y4", hq, sz=8)
                        uC0, uC1 = xpair_w(st['C', q], "xC", hq, sz=8)
                        v = v4[0 if rab == 'a' else 1]
                        mvt = T(f"mv{i}", hq, 8)
                        nc.vector.tensor_tensor(mvt[:], F_om[:, slq, 2 + ki],
                                                v[:], ALU.mult)
                        nc.vector.tensor_tensor(Fal[:, slq, 2 * i], mvt[:],
                                                uC0[:], ALU.mult)
                        nc.vector.tensor_tensor(Fal[:, slq, 2 * i + 1], mvt[:],
                                                uC1[:], ALU.mult)
                    continue
                v = tapv[ki][0 if rab == 'a' else 1] if tapv[ki] else None
                u0, u1 = tapu[TAP_PAIR[ki]]
                m_ap = F_om[:, sl, 2 + ki]
                if v is None:  # tap8: v == 1
                    mv = m_ap
                else:
                    mvt = T(f"mv{i}", h)
                    nc.vector.tensor_tensor(mvt[:], m_ap, v[:], ALU.mult)
                    mv = mvt[:]
                nc.vector.tensor_tensor(Fal[:, sl, 2 * i], mv, u0[:], ALU.mult)
                nc.vector.tensor_tensor(Fal[:, sl, 2 * i + 1], mv, u1[:],
                                        ALU.mult)

        def idx_fb(fb):
            # PE permutation-matmul moves the pixel dim into the free dim, so
            # the (q=p%16, r=p//16) interleave of the gather idx layout becomes
            # DMA-expressible with <=3-dim APs.
            pidx = ps_idx.tile([128, 128], F32, name="pidx", tag="pidx")
            nc.tensor.matmul(pidx[:], Fidx2[:, 2 * fb:2 * fb + 2, :],
                             perm[:], start=True, stop=True)
            nc.vector.tensor_copy(TpI[:, fb, :], pidx[:])
            for ch in (2 * fb, 2 * fb + 1):
                src = TpI[64 * (ch % 2):64 * (ch % 2) + 36, ch // 2, :]
                nc.sync.dma_start(
                    bass.AP(tensor=d4, offset=ch * 4608,
                            ap=[[8, 36], [288, 16], [1, 8]]),
                    src.rearrange("p (q r) -> p q r", r=8))
            for ch in (2 * fb, 2 * fb + 1):
                # ACT's HWDGE ring, so these pipeline with SP-issued d4 builds
                nc.scalar.dma_start(
                    idx_sb[:, ch, :],
                    bass.AP(tensor=d4, offset=ch * 4608,
                            ap=[[0, 8], [288, 16], [1, 288]]))

        # ---- om + coords + idx, pipelined per half-core; gather idxs only
        # need the index path, so alphas (and the mask sigmoid) come after ----
        for h in range(2):
            st = coords_idx(h)
            for q in range(2):
                om_chunk(4 * h + 2 * q)
                om_chunk(4 * h + 2 * q + 1)
                coords_idx_om(h, st, q)
                idx_fb(2 * h + q)
            # masks live in the free dim -> sigmoid is legal here
            nc.scalar.activation(F_om[:, GH * h:GH * (h + 1), 2:7],
                                 F_om[:, GH * h:GH * (h + 1), 2:7],
                                 ACTF.Sigmoid)
            coords_alpha(h, st)

        # ---- main per-chunk pipeline ----
        gather_src = bass.AP(tensor=clast_h, offset=0,
                             ap=[[256, NSRC - 1], [1, 512]])
        for ch in range(NCHUNK):
            gta = pg.tile([128, 20, 512], GDT, name="gta", tag="gta")
            gtb = pg.tile([128, 16, 512], GDT, name="gtb", tag="gtb")
            nc.gpsimd.dma_gather(
                gta[:], gather_src, idx_sb[:, ch, 0:160], 2560, 2560, 512,
                elem_step=256, single_packet=False)
            nc.gpsimd.dma_gather(
                gtb[:], gather_src, idx_sb[:, ch, 160:288], 2048, 2048, 512,
                elem_step=256, single_packet=False)

            # combine via diag-scaled PE matmuls: for each tap/ct, accumulate
            # S^T[c', px] = sum_corners G_corner^T @ diag(alpha_corner) in PSUM
            sc = psc.tile([128, 5, 2, 512], BF16, name="sc", tag="sc")
            for ki in range(5):
                corners = TAP_CORNERS[ki]
                # diag(alpha) tiles, one per (gg, corner), shared by both ct
                dgs = {}
                for gg in range(4):
                    for cidx, (i, s) in enumerate(corners):
                        dg = pdiag.tile([128, 128], BF16, name="dg", tag="dg")
                        asl = Fal[:, ch * 4 + gg, 2 * i + s:2 * i + s + 1]
                        nc.vector.tensor_scalar(dg[:], ident[:], asl, None,
                                                ALU.mult)
                        dgs[gg, cidx] = dg
                for ct in range(2):
                    psS = ps_S.tile([128, 4, 128], F32, name="psS", tag="psS")
                    for gg in range(4):
                        for cidx, (i, s) in enumerate(corners):
                            if i < 5:
                                gsl = gta[:, i * 4 + gg,
                                          s * 256 + ct * 128:
                                          s * 256 + ct * 128 + 128]
                            else:
                                gsl = gtb[:, (i - 5) * 4 + gg,
                                          s * 256 + ct * 128:
                                          s * 256 + ct * 128 + 128]
                            nc.tensor.matmul(
                                psS[:, gg, :], gsl, dgs[gg, cidx][:],
                                start=(cidx == 0),
                                stop=(cidx == len(corners) - 1))
                    nc.scalar.copy(sc[:, ki, ct, :],
                                   psS[:].rearrange("p a b -> p (a b)"))

            # einsum + relu
            feat = pfeat.tile([128, 2, 512], BF16, name="feat", tag="feat")
            for ot in range(2):
                psf = ps_f.tile([128, 512], F32, name="psf", tag="psf")
                n = 0
                for ki in range(5):
                    for ct in range(2):
                        nc.tensor.matmul(
                            psf[:], wmain_sb[:, ki, ct, ot, :],
                            sc[:, ki, ct, :],
                            start=(n == 0), stop=(n == 9))
                        n += 1
                nc.scalar.activation(feat[:, ot, :], psf[:], ACTF.Relu,
                                     bias=bmain_sb[:, ot:ot + 1])
            # 1x1 conv
            pso1 = ps_small.tile([2, 512], F32, name="pso1", tag="pso")
            for ot in range(2):
                nc.tensor.matmul(pso1[:], w1_sb[:, ot, :], feat[:, ot, :],
                                 start=(ot == 0), stop=(ot == 1))
            osb = pout.tile([2, 512], F32, name="osb", tag="osb")
            nc.scalar.activation(osb[:], pso1[:], ACTF.Identity, bias=b1_sb[:])
            nc.sync.dma_start(
                bass.AP(tensor=out,
                        offset=ch * 512, ap=[[RPC * 128, 2], [1, 512]]),
                osb[:])
    return nc


_bf = ml_dtypes.bfloat16


_f8 = ml_dtypes.float8_e4m3fn
_gdt = _f8 if QFP8 else _bf


def _prep_shared(w_om, b_om, weight, bias, w_1x1, b_1x1):
    if QFP8:
        # womT [c', (dy,dx), ct, ch] = 64*w_om[ch, ct*128+c', dy, dx]
        womT = np.ascontiguousarray(
            (w_om * OM_WSCALE).reshape(7, 2, 128, 3, 3)
            .transpose(2, 3, 4, 1, 0).reshape(128, 9, 2, 7)).astype(_f8)
    else:
        # womT [c', t=(dy,dx,ct), ch] = w_om[ch, ct*128+c', dy, dx]
        womT = np.ascontiguousarray(
            w_om.reshape(7, 2, 128, 3, 3).transpose(2, 3, 4, 1, 0)
            .reshape(128, 18, 7)).astype(_bf)
    wr = weight.reshape(O, C, 9)[:, :, [0, 2, 4, 6, 8]]  # [O, C, 5]
    # wmainT [c', ki, ct, ot, o'] = wr[ot*128+o', ct*128+c', ki]
    wmainT = np.ascontiguousarray(
        wr.reshape(2, 128, 2, 128, 5).transpose(3, 4, 2, 0, 1)).astype(_bf)
    w1 = w_1x1[:, :, 0, 0]  # [F, O]
    # w1T [o', ot, f] = w1[f, ot*128+o']
    w1T = np.ascontiguousarray(
        w1.reshape(2, 2, 128).transpose(2, 1, 0)).astype(_bf)
    return {
        "womT": womT,
        "wmainT": wmainT,
        "w1x1T": w1T,
        "bom": b_om.reshape(7, 1).astype(np.float32),
        "bmain": np.ascontiguousarray(
            bias.reshape(2, 128).T).astype(np.float32),
        "b1": b_1x1.reshape(2, 1).astype(np.float32),
    }


def kernel(inp, wh, w_om, b_om, weight, bias, w_1x1, b_1x1):
    inp = np.asarray(inp, np.float32)
    wh = np.asarray(wh, np.float32)
    shared = _prep_shared(np.asarray(w_om, np.float32),
                          np.asarray(b_om, np.float32),
                          np.asarray(weight, np.float32),
                          np.asarray(bias, np.float32),
                          np.asarray(w_1x1, np.float32),
                          np.asarray(b_1x1, np.float32))
    in_maps = []
    for core in range(NCORES):
        b = core // (NCORES // B)
        r0 = (core % (NCORES // B)) * RPC
        lo, hi = r0 - HALO, r0 + RPC + HALO
        plo, phi = max(0, -lo), max(0, hi - H)
        sl = inp[b, :, max(0, lo):min(H, hi)]
        sl = np.pad(sl, ((0, 0), (plo, phi), (0, 0)))  # [256, RTOT, 128]
        slb = sl.astype(_gdt)
        m = dict(shared)
        m["xinb"] = np.ascontiguousarray(slb.reshape(2, 128, RTOT, 128))
        # channels-last gather source: clast[px, ch], px = r*128 + x
        m["clast"] = np.ascontiguousarray(
            slb.transpose(1, 2, 0).reshape(NSRC, 256))
        m["whin"] = np.ascontiguousarray(wh[b, :, r0:r0 + RPC])
        m["ygl"] = (r0 + np.arange(G, dtype=np.float32)).reshape(1, G)
        m["rbase"] = np.array([[r0]], np.float32)
        in_maps.append(m)

    nc = bacc.Bacc("TRN2")
    build(nc)
    nc.finalize()  # Bacc.compile(): legalizes sync waits (1 per instruction)
    res = run_bass_kernel_spmd(nc, in_maps, list(range(NCORES)),
                               trace=bool(int(os.environ.get("KTRACE", "0"))))
    outp = np.empty((B, F, H, W), np.float32)
    for core in range(NCORES):
        b = core // (NCORES // B)
        r0 = (core % (NCORES // B)) * RPC
        outp[b, :, r0:r0 + RPC] = res.results[core]["out"]
    kernel.last_results = res
    return outp

